# revision 76
# baseline (speedup 1.0000x reference)
"""Trainium2 Bass kernel for the stacked relu-LSTM classifier pair.

Strategy (pure data parallel over 8 NeuronCores):
  - Each core processes B/8 = 8192 samples; weights replicated.
  - On-chip layout is feature-major ([features, samples]) so the LSTM
    recurrence never needs per-step transposes; inputs are transposed
    once on entry via PE-transpose (bf16), softmax tail runs batch-major
    via a stationary-activation matmul trick.
  - Matmuls/activations in bf16, PSUM accumulation in fp32, outputs fp32.
"""

import numpy as np
import ml_dtypes

import concourse.bass as bass
import concourse.mybir as mybir
import concourse.tile as tile
from concourse import bacc
from concourse.bass import ds, ts
from concourse.bass_utils import run_bass_kernel_spmd
from concourse.masks import make_identity

F32 = mybir.dt.float32
BF16 = mybir.dt.bfloat16
AF = mybir.ActivationFunctionType
ALU = mybir.AluOpType

N_CORES = 8
B_TOTAL = 65536
BC = B_TOTAL // N_CORES  # samples per core
BT = 1024                # samples per B-tile
NCH = BT // 128          # 128-sample chunks per B-tile
F = 132                  # input features
FA, FR = 128, 4          # main/remainder feature split


def _cell_u64(nc, pools, ga, gb, bias, c_t, h_out, t, add_on_pool=False):
    """LSTM cell for u=64 layers. ga=[i;f] gb=[g;o] psum fp32 [128,BT].
    bias fp32 [128,2]. c_t [128,BT] bf16 tile, state in rows 64:128 (matches
    the f-gate rows so SBUF+SBUF operands share a base partition; walrus
    requires equal bases for two-SBUF-input ops). h_out AP [64,BT] bf16."""
    sig128, sig64, cell = pools
    bt = ga.shape[-1]
    cs = c_t[64:128, :]
    s_if = sig128.tile([128, bt], BF16, tag="sig128")
    nc.scalar.activation(s_if, ga, AF.Sigmoid, bias=bias[:, 0:1])
    s_o = sig64.tile([128, bt], BF16, tag="sig64")
    nc.scalar.activation(s_o[64:128, :], gb[64:128, :], AF.Sigmoid,
                         bias=bias[64:128, 1:2])
    # ig = relu(g) * sigmoid(i); at t=0 c = ig directly (c_prev = 0)
    tgt = cs if t == 0 else cell.tile([64, bt], BF16, tag="cell")
    nc.vector.scalar_tensor_tensor(
        out=tgt, in0=gb[0:64, :], scalar=0.0, in1=s_if[0:64, :],
        op0=ALU.max, op1=ALU.mult)
    if t > 0:
        fc = cell.tile([64, bt], BF16, tag="cell")
        nc.gpsimd.tensor_mul(fc, s_if[64:128, :], cs)
        if add_on_pool:
            nc.gpsimd.tensor_add(cs, tgt, fc)
        else:
            nc.vector.tensor_add(cs, tgt, fc)
    # h = relu(c) * sigmoid(o)
    nc.vector.scalar_tensor_tensor(
        out=h_out, in0=cs, scalar=0.0, in1=s_o[64:128, :],
        op0=ALU.max, op1=ALU.mult)


def build(nc_samples=BC, bt=BT, n_cores=N_CORES):
    """Build the per-core Bass program (same program on all cores)."""
    nt = nc_samples // bt
    nch = bt // 128
    nsub = bt // 512  # 512-wide matmul subtiles per B-tile

    nc = bacc.Bacc(
        "TRN2", target_bir_lowering=False, debug=False,
        enable_asserts=False, num_devices=n_cores,
    )

    # ---- DRAM I/O ----
    x_ac = nc.dram_tensor("x_ac", [nc_samples, 3 * F], F32, kind="ExternalInput")
    x_cc = nc.dram_tensor("x_cc", [nc_samples, 5 * F], F32, kind="ExternalInput")
    wd = {}
    for name, shape in [
        ("w1m", [FA, 256]),
        ("u1a0", [64 + 3 * FR, 256]), ("u1a1", [64 + 3 * FR, 256]),
        ("u1a2", [64 + 3 * FR, 256]),
        ("w2", [64, 512]), ("u2", [128, 512]),
        ("w3", [128, 256]), ("u3", [64, 256]),
        ("wd1", [64, 100]), ("wd2a", [101, 11]),
        ("wccm", [FA, 256]),
        ("ucca0", [64 + 5 * FR, 256]), ("ucca1", [64 + 5 * FR, 256]),
        ("ucca2", [64 + 5 * FR, 256]), ("ucca3", [64 + 5 * FR, 256]),
        ("ucca4", [64 + 5 * FR, 256]),
        ("wdcca", [65, 10]),
    ]:
        wd[name] = nc.dram_tensor(name, shape, BF16, kind="ExternalInput")
    for name, shape in [
        ("b1", [128, 2]), ("b2", [128, 4]), ("b3", [128, 2]),
        ("bcc", [128, 2]), ("bd1", [100, 1]),
    ]:
        wd[name] = nc.dram_tensor(name, shape, F32, kind="ExternalInput")
    o_ac = nc.dram_tensor("o_ac", [nc_samples, 11], F32, kind="ExternalOutput")
    o_cc = nc.dram_tensor("o_cc", [nc_samples, 10], F32, kind="ExternalOutput")

    with tile.TileContext(nc) as tc:
        with (
            tc.tile_pool(name="const", bufs=1) as const,
            tc.tile_pool(name="xin", bufs=3) as xin,
            tc.tile_pool(name="xconv", bufs=5) as xconv,
            tc.tile_pool(name="xt", bufs=3) as xtp,
            tc.tile_pool(name="hseq", bufs=2) as hseq,
            tc.tile_pool(name="sig", bufs=6) as sig128,
            tc.tile_pool(name="sig64", bufs=4) as sig64,
            tc.tile_pool(name="cellp", bufs=4) as cellp,
            tc.tile_pool(name="cst", bufs=2) as cst,
            tc.tile_pool(name="dense", bufs=1) as dense,
            tc.tile_pool(name="smax", bufs=2) as smax,
            tc.tile_pool(name="pg", bufs=3, space="PSUM") as pg,
            tc.tile_pool(name="ptr", bufs=2, space="PSUM") as ptr,
        ):
            # ---- constants ----
            ident = const.tile([128, 128], BF16)
            make_identity(nc, ident)
            W = {}
            for name in wd:
                if name.startswith("x") or name.startswith("o_"):
                    continue
                t_ = const.tile(wd[name].shape, wd[name].dtype, tag=name)
                nc.sync.dma_start(out=t_, in_=wd[name][:, :])
                W[name] = t_
            # dense-2 activations with a trailing ones row (bias via matmul);
            # partition starts must be 32-aligned, so the ones live at 96+/64+
            # and the activation writer overlaps rows below.
            a1s, accs = [], []
            for k in range(2):
                a1_ = dense.tile([128, bt], BF16, tag=f"a1_{k}")
                nc.gpsimd.memset(a1_[96:128, :], 1.0)
                a1s.append(a1_)
                acc_ = dense.tile([128, bt], BF16, tag=f"acc_{k}")
                nc.gpsimd.memset(acc_[64:128, :], 1.0)
                accs.append(acc_)

            cellpools = (sig128, sig64, cellp)

            def emit_tile(ib):
                b0 = ib * bt
                a1 = a1s[ib % 2]
                acc = accs[ib % 2]
                # ================= input stage =================
                # load fp32 -> bf16 convert on Pool -> XBAR DMA-transpose
                # straight into SBUF (no PSUM, no PE, no evac compute).
                # Remainder features (128:132 per step) of both branches ride
                # one padded [128,128] staging tile per chunk: cols 0:12 hold
                # the ac rem (t-major), cols 32:52 the cc rem.
                xT_a = xtp.tile([128, 4, bt], BF16, tag="xTa")
                xT_c = xtp.tile([128, 6, bt], BF16, tag="xTc")
                for c in range(nch):
                    r0 = b0 + c * 128
                    csl = ds(c * 128, 128)
                    xa = xin.tile([128, 3 * F], F32, tag="xa")
                    nc.sync.dma_start(out=xa, in_=x_ac[r0:r0 + 128, :])
                    xa3 = xa.rearrange("p (t f) -> p t f", t=3)
                    xab = xconv.tile([128, 3, FA], BF16, tag="xab")
                    nc.gpsimd.tensor_copy(out=xab, in_=xa3[:, :, 0:FA])
                    xar = xconv.tile([128, 3 * FR], BF16, tag="xar")
                    nc.gpsimd.tensor_copy(
                        out=xar.rearrange("p (t f) -> p t f", f=FR),
                        in_=xa3[:, :, FA:F])
                    pst_a = ptr.tile([128, 4, 128], BF16, tag="tr")
                    for t in range(3):
                        nc.tensor.transpose(
                            pst_a[:, t, :], xab[:, t, :], ident)
                    nc.tensor.transpose(
                        pst_a[0:3 * FR, 3, 0:128], xar, ident)
                    nc.vector.tensor_copy(
                        out=xT_a[:, 0:3, csl], in_=pst_a[:, 0:3, :])
                    nc.vector.tensor_copy(
                        out=xT_a[0:3 * FR, 3, csl], in_=pst_a[0:3 * FR, 3, :])

                    xc = xin.tile([128, 5 * F], F32, tag="xc")
                    nc.sync.dma_start(out=xc, in_=x_cc[r0:r0 + 128, :])
                    xc5 = xc.rearrange("p (t f) -> p t f", t=5)
                    xcb = xconv.tile([128, 5, FA], BF16, tag="xcb")
                    nc.gpsimd.tensor_copy(out=xcb, in_=xc5[:, :, 0:FA])
                    xcr = xconv.tile([128, 5 * FR], BF16, tag="xcr")
                    nc.gpsimd.tensor_copy(
                        out=xcr.rearrange("p (t f) -> p t f", f=FR),
                        in_=xc5[:, :, FA:F])
                    pst_c = ptr.tile([128, 6, 128], BF16, tag="tr")
                    for t in range(5):
                        nc.tensor.transpose(
                            pst_c[:, t, :], xcb[:, t, :], ident)
                    nc.tensor.transpose(
                        pst_c[0:5 * FR, 5, 0:128], xcr, ident)
                    nc.scalar.copy(
                        out=xT_c[:, 0:5, csl], in_=pst_c[:, 0:5, :])
                    nc.scalar.copy(
                        out=xT_c[0:5 * FR, 5, csl], in_=pst_c[0:5 * FR, 5, :])

                # ===== both branches, emitted as interleaved t-steps so the
                # static per-engine order staggers two independent chains =====
                h1s = hseq.tile([64 + 3 * FR, 4, bt], BF16, tag="h1s")
                for s in range(3):
                    nc.vector.tensor_copy(
                        out=h1s[64:64 + 3 * FR, s, :], in_=xT_a[0:3 * FR, 3, :])
                c1 = cst.tile([128, bt], BF16, tag="c1")
                h2s = hseq.tile([128, 3, bt], BF16, tag="h2s")
                c2 = cst.tile([128, bt], BF16, tag="c2")
                c3 = cst.tile([128, bt], BF16, tag="c3")
                hcs = hseq.tile([64 + 5 * FR, 6, bt], BF16, tag="hcs")
                for s in range(5):
                    nc.vector.tensor_copy(
                        out=hcs[64:64 + 5 * FR, s, :], in_=xT_c[0:5 * FR, 5, :])
                c4 = cst.tile([128, bt], BF16, tag="c4")
                st = {"h3": None}

                def rec_lstm_u64(t, wx, xsrc, ua, hs, bias, c_t, T,
                                 add_on_pool=False):
                    ga = pg.tile([128, bt], F32, tag="pg")
                    gb = pg.tile([128, bt], F32, tag="pg")
                    for m, gt in ((0, ga), (1, gb)):
                        for n in range(nsub):
                            nsl = ds(n * 512, 512)
                            nc.tensor.matmul(
                                gt[:, nsl], W[wx][:, ts(m, 128)],
                                xsrc[:, t, nsl], start=True, stop=False)
                        for n in range(nsub):
                            nsl = ds(n * 512, 512)
                            if t == 0:
                                # h_{-1} = 0: only the x-remainder rows feed in
                                nc.tensor.matmul(
                                    gt[:, nsl],
                                    W[f"{ua}0"][64:64 + T * FR, ts(m, 128)],
                                    hs[64:64 + T * FR, 0, nsl],
                                    start=False, stop=True)
                            else:
                                nc.tensor.matmul(
                                    gt[:, nsl], W[f"{ua}{t}"][:, ts(m, 128)],
                                    hs[:, t, nsl], start=False, stop=True)
                    _cell_u64(nc, cellpools, ga, gb, W[bias], c_t,
                              hs[0:64, t + 1, :], t, add_on_pool)

                def l1_step(t):
                    rec_lstm_u64(t, "w1m", xT_a, "u1a", h1s, "b1", c1, 3)

                def l2_step(t):
                    gs = {}
                    for wave in ((0, 1), (2, 3)):
                        for m in wave:
                            g = pg.tile([128, bt], F32, tag="pg")
                            gs[m] = g
                            for n in range(nsub):
                                nsl = ds(n * 512, 512)
                                nc.tensor.matmul(
                                    g[:, nsl], W["w2"][:, ts(m, 128)],
                                    h1s[0:64, t + 1, nsl],
                                    start=True, stop=(t == 0))
                            if t > 0:
                                for n in range(nsub):
                                    nsl = ds(n * 512, 512)
                                    nc.tensor.matmul(
                                        g[:, nsl], W["u2"][:, ts(m, 128)],
                                        h2s[:, t - 1, nsl],
                                        start=False, stop=True)
                        if wave == (0, 1):
                            s_i = sig128.tile([128, bt], BF16, tag="sig128")
                            nc.scalar.activation(
                                s_i, gs[0], AF.Sigmoid, bias=W["b2"][:, 0:1])
                            s_f = sig128.tile([128, bt], BF16, tag="sig128")
                            nc.scalar.activation(
                                s_f, gs[1], AF.Sigmoid, bias=W["b2"][:, 1:2])
                    s_o = sig128.tile([128, bt], BF16, tag="sig128")
                    nc.scalar.activation(
                        s_o, gs[3], AF.Sigmoid, bias=W["b2"][:, 3:4])
                    tgt = c2 if t == 0 else cellp.tile([128, bt], BF16,
                                                       tag="cell")
                    nc.vector.scalar_tensor_tensor(
                        out=tgt, in0=gs[2], scalar=0.0, in1=s_i,
                        op0=ALU.max, op1=ALU.mult)
                    if t > 0:
                        fc = cellp.tile([128, bt], BF16, tag="cell")
                        nc.gpsimd.tensor_mul(fc, s_f, c2)
                        nc.vector.tensor_add(c2, tgt, fc)
                    nc.vector.scalar_tensor_tensor(
                        out=h2s[:, t, :], in0=c2, scalar=0.0, in1=s_o,
                        op0=ALU.max, op1=ALU.mult)

                def l3_step(t):
                    ga = pg.tile([128, bt], F32, tag="pg")
                    gb = pg.tile([128, bt], F32, tag="pg")
                    for m, gt in ((0, ga), (1, gb)):
                        for n in range(nsub):
                            nsl = ds(n * 512, 512)
                            nc.tensor.matmul(
                                gt[:, nsl], W["w3"][:, ts(m, 128)],
                                h2s[:, t, nsl], start=True, stop=(t == 0))
                        if t > 0:
                            for n in range(nsub):
                                nsl = ds(n * 512, 512)
                                nc.tensor.matmul(
                                    gt[:, nsl], W["u3"][:, ts(m, 128)],
                                    st["h3"][:, nsl], start=False, stop=True)
                    h3n = cellp.tile([64, bt], BF16, tag="h3")
                    _cell_u64(nc, cellpools, ga, gb, W["b3"], c3, h3n, t)
                    st["h3"] = h3n

                def cc_step(t):
                    rec_lstm_u64(t, "wccm", xT_c, "ucca", hcs, "bcc", c4, 5)

                def dense_both():
                    # dense1 relu on ACT (Relu shares the sigmoid act-table
                    # set, so no table reload)
                    d1ps = pg.tile([100, bt], F32, tag="pg")
                    for n in range(nsub):
                        nsl = ds(n * 512, 512)
                        nc.tensor.matmul(d1ps[:, nsl], W["wd1"],
                                         st["h3"][:, nsl],
                                         start=True, stop=True)
                    nc.scalar.activation(a1[0:100, :], d1ps, AF.Relu,
                                         bias=W["bd1"][:, 0:1])
                    nc.vector.tensor_copy(out=acc[0:64, :],
                                          in_=hcs[0:64, 5, :])
                    # both branches' logits share one PSUM tile so a single
                    # Exp op covers them (one act-table round trip per tile)
                    dps = pg.tile([128, nch, 21], F32, tag="pg")
                    for c in range(nch):
                        nc.tensor.matmul(
                            dps[:, c, 0:11], a1[0:101, ds(c * 128, 128)],
                            W["wd2a"], start=True, stop=True)
                        nc.tensor.matmul(
                            dps[:, c, 11:21], acc[0:65, ds(c * 128, 128)],
                            W["wdcca"], start=True, stop=True)
                    e_b = smax.tile([128, nch, 21], F32, tag="eb")
                    nc.scalar.activation(e_b, dps, AF.Exp)
                    sums = smax.tile([128, 2, nch], F32, tag="sums")
                    nc.vector.tensor_reduce(
                        sums[:, 0, :], e_b[:, :, 0:11],
                        axis=mybir.AxisListType.X, op=ALU.add)
                    nc.vector.tensor_reduce(
                        sums[:, 1, :], e_b[:, :, 11:21],
                        axis=mybir.AxisListType.X, op=ALU.add)
                    nc.vector.reciprocal(sums, sums)
                    oa = smax.tile([128, nch, 11], F32, tag="oa")
                    rb = sums[:, 0, :]
                    rb = bass.AP(tensor=rb.tensor, offset=rb.offset,
                                 ap=[*rb.ap, [0, 11]])
                    nc.vector.tensor_tensor(out=oa, in0=e_b[:, :, 0:11],
                                            in1=rb, op=ALU.mult)
                    oc = smax.tile([128, nch, 10], F32, tag="oc")
                    rb = sums[:, 1, :]
                    rb = bass.AP(tensor=rb.tensor, offset=rb.offset,
                                 ap=[*rb.ap, [0, 10]])
                    nc.vector.tensor_tensor(out=oc, in0=e_b[:, :, 11:21],
                                            in1=rb, op=ALU.mult)
                    nc.scalar.dma_start(
                        out=o_ac[b0:b0 + bt, :].rearrange(
                            "(c p) f -> p c f", p=128),
                        in_=oa)
                    nc.scalar.dma_start(
                        out=o_cc[b0:b0 + bt, :].rearrange(
                            "(c p) f -> p c f", p=128),
                        in_=oc)

                steps_ac = ([lambda t=t: l1_step(t) for t in range(3)]
                            + [lambda t=t: l2_step(t) for t in range(3)]
                            + [lambda t=t: l3_step(t) for t in range(3)]
                            + [dense_both])
                steps_cc = [lambda t=t: cc_step(t) for t in range(5)]
                order = []
                ia = ic = 0
                while ia < len(steps_ac) or ic < len(steps_cc):
                    if ia < len(steps_ac):
                        order.append(steps_ac[ia]); ia += 1
                    if ic < len(steps_cc):
                        order.append(steps_cc[ic]); ic += 1
                return order

            # emit tiles in pairs, steps woven, so the static per-engine
            # order always carries two independent tiles' chains
            for p0 in range(0, nt, 2):
                s0 = emit_tile(p0)
                s1 = emit_tile(p0 + 1) if p0 + 1 < nt else []
                i = j = 0
                while i < len(s0) or j < len(s1):
                    if i < len(s0):
                        s0[i](); i += 1
                    if j < len(s1):
                        s1[j](); j += 1

    nc.compile()
    return nc


def prep_weights(inp):
    """Host-side weight preprocessing -> per-core input map (minus x shards)."""
    bf = ml_dtypes.bfloat16
    f32 = np.float32

    def check_zero(v, what):
        assert not np.any(np.asarray(v)), f"nonzero {what} unsupported"

    W1, U1, b1 = (np.asarray(inp[k], f32) for k in ("W1", "U1", "b1"))
    W2, U2, b2 = (np.asarray(inp[k], f32) for k in ("W2", "U2", "b2"))
    W3, U3, b3 = (np.asarray(inp[k], f32) for k in ("W3", "U3", "b3"))
    Wd1, bd1 = np.asarray(inp["Wd1"], f32), np.asarray(inp["bd1"], f32)
    Wd2, bd2 = np.asarray(inp["Wd2"], f32), np.asarray(inp["bd2"], f32)
    Wcc, Ucc, bcc = (np.asarray(inp[k], f32) for k in ("Wcc", "Ucc", "bcc"))
    Wdcc, bdcc = np.asarray(inp["Wdcc"], f32), np.asarray(inp["bdcc"], f32)

    # relu-gate bias must be zero (it is in this model); sigmoid biases are
    # applied on the ACT engine, dense biases via ones-row matmul trick.
    check_zero(b1[128:192], "b1 g-gate bias")
    check_zero(b2[256:384], "b2 g-gate bias")
    check_zero(b3[128:192], "b3 g-gate bias")
    check_zero(bcc[128:192], "bcc g-gate bias")

    def masked_u(U, Wrem, T, t):
        # [64 + T*FR, 4u]: U on top; W[128:132] in block-row t, zeros elsewhere
        out = np.zeros((64 + T * FR, U.shape[1]), np.float32)
        out[0:64] = U
        out[64 + t * FR:64 + (t + 1) * FR] = Wrem
        return out.astype(bf)

    m = {
        "w1m": W1[:FA].astype(bf),
        **{f"u1a{t}": masked_u(U1, W1[FA:], 3, t) for t in range(3)},
        **{f"ucca{t}": masked_u(Ucc, Wcc[FA:], 5, t) for t in range(5)},
        "w2": W2.astype(bf), "u2": U2.astype(bf),
        "w3": W3.astype(bf), "u3": U3.astype(bf),
        "wd1": Wd1.astype(bf),
        "wd2a": np.concatenate([Wd2, bd2[None, :]], 0).astype(bf),
        "wccm": Wcc[:FA].astype(bf),
        "wdcca": np.concatenate([Wdcc, bdcc[None, :]], 0).astype(bf),
        "b1": np.ascontiguousarray(b1.reshape(2, 128).T),
        "b2": np.ascontiguousarray(b2.reshape(4, 128).T),
        "b3": np.ascontiguousarray(b3.reshape(2, 128).T),
        "bcc": np.ascontiguousarray(bcc.reshape(2, 128).T),
        "bd1": np.ascontiguousarray(bd1.reshape(100, 1)),
    }
    return m


_NC_CACHE = {}


def _get_nc(nc_samples=BC, bt=BT, n_cores=N_CORES):
    key = (nc_samples, bt, n_cores)
    if key not in _NC_CACHE:
        _NC_CACHE[key] = build(nc_samples, bt, n_cores)
    return _NC_CACHE[key]


def _run(inputs, trace=False):
    nc = _get_nc()
    wm = prep_weights(inputs)
    xa = np.ascontiguousarray(
        np.asarray(inputs["input_ac"], np.float32).reshape(B_TOTAL, 3 * F))
    xc = np.ascontiguousarray(
        np.asarray(inputs["input_cc"], np.float32).reshape(B_TOTAL, 5 * F))
    in_maps = []
    for i in range(N_CORES):
        im = dict(wm)
        im["x_ac"] = np.ascontiguousarray(xa[i * BC:(i + 1) * BC])
        im["x_cc"] = np.ascontiguousarray(xc[i * BC:(i + 1) * BC])
        in_maps.append(im)
    res = run_bass_kernel_spmd(nc, in_maps, list(range(N_CORES)), trace=trace)
    out_ac = np.concatenate([r["o_ac"] for r in res.results], 0)
    out_cc = np.concatenate([r["o_cc"] for r in res.results], 0)
    return (out_ac, out_cc), res


def kernel(**inputs):
    out, _ = _run(inputs, trace=False)
    return out
